# revision 1
# baseline (speedup 1.0000x reference)
"""Trainium2 Bass kernel for nn_BehaviorSnake: one CA step on a [B,C,H,W] world.

Sharding: batch-parallel, world[b] -> core b (B == n_cores == 8).

Per-core layout: each [512,512] plane lives in SBUF as [128 partitions, 4, 512]
where block t holds rows 128t..128t+127 (partition p = row 128t+p).
 - W-shifts (torus roll along axis 3) = free-dim offset copies (GPSIMD), or
   folded directly into a consumer's read AP when single-use.
 - H-shifts (roll along axis 2) = SBUF->SBUF partition-offset DMAs on the
   HWDGE queue (engine-free) + small boundary-row DMAs for the torus wrap.
 - Mask algebra in bf16 (exact for {0,1} masks and dir values 0..3), mostly
   on VectorE (tensor_scalar 4x mode + tensor_tensor 2x mode), with the
   blocked*dbta products and a few single-input ops on GPSIMD; compares that
   feed copy_predicated produce uint8 masks (HW requirement).
 - Every plane op is emitted as two half-plane ops (Split proxies) so
   dependent chains pipeline at half-plane granularity.
 - All HBM DMAs are f32 on the HWDGE (SP) queue (engine-free); dtype casts
   are explicit ScalarE copies. Loads are ordered critical-path-first and
   the zero-channel/wall stores are emitted last (lowest DMA priority).
 - f32 kept only where bit-exactness requires it (rand compares, energy).

Channels: 0=EMPTY 1=WALL 2=ACID 3=SNAKE 6=DIR 7=ENERGY; 4,5,8,9 always zero.
"""

import numpy as np
import ml_dtypes

import concourse.bacc as bacc
import concourse.mybir as mybir
import concourse.tile as tile
from concourse import bass_utils

OP = mybir.AluOpType
AF = mybir.ActivationFunctionType
DTB = mybir.dt.bfloat16
DTF = mybir.dt.float32
DTU8 = mybir.dt.uint8

B, C, H, W = 8, 10, 512, 512
NCORES = 8


def make_shmat() -> np.ndarray:
    """4 concatenated 128x128 lhsT matrices for H-shifts, as [128, 512] bf16.

    matmul computes out[m,n] = sum_k lhsT[k,m] * rhs[k,n].
    UP   (out[h] = in[h-1]):  main lhsT = eye(k=1); corner lhsT[127,0]=1,
                              corner rhs = previous block.
    DOWN (out[h] = in[h+1]):  main lhsT = eye(k=-1); corner lhsT[0,127]=1,
                              corner rhs = next block.
    """
    m = np.zeros((128, 512), np.float32)
    m[:, 0:128] = np.eye(128, k=1)
    m[127, 128 + 0] = 1.0
    m[:, 256:384] = np.eye(128, k=-1)
    m[0, 384 + 127] = 1.0
    return m.astype(ml_dtypes.bfloat16)


def snake_body(tc, outs, ins):
    nc = tc.nc
    world = ins["world"]
    rm = ins["rm"]
    re = ins["re"]
    shmat_d = ins["shmat"]
    out = outs["out"]

    def rp(x):
        return x.rearrange("(t p) w -> p t w", p=128)

    from concourse.bass import AP as _AP

    # Engine proxies that split ops over [128, 4, 512]-shaped APs into two
    # half-plane ops (blocks 0-1 / 2-3) so dependent chains pipeline at half-
    # plane granularity instead of serializing on whole planes.
    _SPLIT = {
        "tensor_mul",
        "tensor_add",
        "tensor_sub",
        "tensor_max",
        "tensor_copy",
        "tensor_scalar",
        "tensor_single_scalar",
        "tensor_scalar_mul",
        "tensor_scalar_add",
        "copy_predicated",
        "copy",
        "activation",
        "dma_start",
    }

    class Split:
        def __init__(self, eng):
            self._e = eng

        def __getattr__(self, name):
            f = getattr(self._e, name)
            if name not in _SPLIT:
                return f

            def g(*args, **kw):
                did = False

                def cut(x, sl):
                    nonlocal did
                    if (
                        isinstance(x, _AP)
                        and len(x.shape) == 3
                        and x.shape[1] == 4
                    ):
                        did = True
                        return x[:, sl]
                    return x

                for sl in (slice(0, 2), slice(2, 4)):
                    a2 = [cut(x, sl) for x in args]
                    k2 = {k: cut(v, sl) for k, v in kw.items()}
                    f(*a2, **k2)
                    if not did:
                        return  # nothing split; single full call was emitted

            return g

    V = Split(nc.vector)
    P = Split(nc.gpsimd)
    A = Split(nc.scalar)
    T = nc.tensor
    SY = Split(nc.sync)

    SHP = [128, 4, 512]

    with (
        tc.tile_pool(name="mp", bufs=1) as mp,
        tc.tile_pool(name="shp", bufs=2) as shp,
    ):
        shmat = mp.tile([128, 512], DTB, name="shmat_sb")
        SY.dma_start(out=shmat[:, :], in_=shmat_d)
        UPm = shmat[:, 0:128]
        UPc = shmat[:, 128:256]
        DNm = shmat[:, 256:384]
        DNc = shmat[:, 384:512]

        def hshift(nm, src, up):
            # Torus roll along H via SBUF->SBUF DMA: bulk partition-offset copy
            # + boundary row from the neighboring block (engine-free on HWDGE).
            d = shp.tile(SHP, DTB, tag="hscopy", name=nm, bufs=2)
            if up:  # out[h] = in[h-1]
                SY.dma_start(out=d[1:128, :, :], in_=src[0:127, :, :])
                SY.dma_start(out=d[0:1, 1:4, :], in_=src[127:128, 0:3, :])
                SY.dma_start(out=d[0:1, 0:1, :], in_=src[127:128, 3:4, :])
            else:  # out[h] = in[h+1]
                SY.dma_start(out=d[0:127, :, :], in_=src[1:128, :, :])
                SY.dma_start(out=d[127:128, 0:3, :], in_=src[0:1, 1:4, :])
                SY.dma_start(out=d[127:128, 3:4, :], in_=src[0:1, 0:1, :])
            return d

        def wshift(nm, src, plus):
            d = shp.tile(SHP, DTB, tag="wshift", name=nm, bufs=2)
            if plus:  # out[w] = in[w+1] (shift dir 0)
                P.tensor_copy(out=d[:, :, 0:511], in_=src[:, :, 1:512])
                P.tensor_copy(out=d[:, :, 511:512], in_=src[:, :, 0:1])
            else:  # out[w] = in[w-1] (shift dir 4)
                P.tensor_copy(out=d[:, :, 1:512], in_=src[:, :, 0:511])
                P.tensor_copy(out=d[:, :, 0:1], in_=src[:, :, 511:512])
            return d

        # ---- loads (all f32 via HWDGE; engine-free), critical-path first.
        # The three loop-critical planes load + cast at quarter-plane
        # granularity so the first compute ops start as early as possible.
        S0_f = shp.tile(SHP, DTF, tag="f32s", name="S0_f", bufs=3)
        D0_f = shp.tile(SHP, DTF, tag="f32s", name="D0_f", bufs=3)
        Wl_f = shp.tile(SHP, DTF, tag="f32s", name="Wl_f", bufs=3)
        S0b = mp.tile(SHP, DTB, name="S0b")
        D0b = mp.tile(SHP, DTB, name="D0b")
        Wlb = mp.tile(SHP, DTB, name="Wlb")
        for src_c, ftile, btile in (
            (3, S0_f, S0b),
            (6, D0_f, D0b),
            (1, Wl_f, Wlb),
        ):
            for t in range(4):
                SY.dma_start(
                    out=ftile[:, t : t + 1, :], in_=rp(world[src_c])[:, t : t + 1, :]
                )
                A.copy(btile[:, t : t + 1, :], ftile[:, t : t + 1, :])
        Re = shp.tile(SHP, DTF, tag="f32s", name="Re", bufs=3)
        SY.dma_start(out=Re[:, :, :], in_=rp(re))
        Rm = shp.tile(SHP, DTF, tag="f32s", name="Rm", bufs=3)
        SY.dma_start(out=Rm[:, :, :], in_=rp(rm))


        # ---- preamble compute ----
        t_acc = mp.tile(SHP, DTB, name="t_acc")  # does_turn accum
        V.tensor_single_scalar(t_acc[:, :, :], Rm[:, :, :], 0.1, OP.is_lt)
        lt05b = mp.tile(SHP, DTB, name="lt05b")
        V.tensor_single_scalar(lt05b[:, :, :], Re[:, :, :], 0.05, OP.is_lt)
        q2 = mp.tile(SHP, DTB, name="q2")  # {-2, 0} = -2*(Re<0.5)
        V.tensor_scalar(q2[:, :, :], Re[:, :, :], 0.5, -2.0, OP.is_lt, OP.mult)
        ws0 = mp.tile(SHP, DTB, name="ws0")  # wall|snake (angles 0,1)
        V.tensor_add(ws0[:, :, :], Wlb[:, :, :], S0b[:, :, :])
        notWl = mp.tile(SHP, DTB, name="notWl")
        V.tensor_scalar(notWl[:, :, :], Wlb[:, :, :], -1.0, 1.0, OP.mult, OP.add)
        A0_f = shp.tile(SHP, DTF, tag="f32s", name="A0_f", bufs=3)
        SY.dma_start(out=A0_f[:, :, :], in_=rp(world[2]))
        A0b = mp.tile(SHP, DTB, name="A0b")
        A.copy(A0b[:, :, :], A0_f[:, :, :])
        En0 = mp.tile(SHP, DTF, name="En0")
        SY.dma_start(out=En0[:, :, :], in_=rp(world[7]))
        epos = mp.tile(SHP, DTB, name="epos")  # (En0 > 0), exact since En0 >= 0
        A.activation(epos[:, :, :], En0[:, :, :], AF.Sign)
        E0c = shp.tile(SHP, DTB, tag="E0c", name="E0c", bufs=1)  # original EMPTY
        P.tensor_scalar(E0c[:, :, :], ws0[:, :, :], -1.0, 1.0, OP.mult, OP.add)
        P.tensor_sub(E0c[:, :, :], E0c[:, :, :], A0b[:, :, :])
        em = shp.tile(SHP, DTF, tag="f32s", name="em", bufs=3)  # En0 - 0.1
        P.tensor_single_scalar(em[:, :, :], En0[:, :, :], 0.1, OP.subtract)

        def isa(a):
            d = shp.tile(SHP, DTB, tag="isa", name=f"isa{a}", bufs=1)
            V.tensor_single_scalar(d[:, :, :], D0b[:, :, :], float(a), OP.is_equal)
            return d

        # ---- angle 0 (dbta dir4 = W-, blocked dir0 = W+) ----
        isa0 = isa(0)
        msa0 = shp.tile(SHP, DTB, tag="msa", name="msa0", bufs=2)
        V.tensor_mul(msa0[:, :, :], isa0[:, :, :], S0b[:, :, :])
        shm0 = wshift("shm0", msa0, plus=False)  # dbta0 (unmasked)
        bd0 = shp.tile(SHP, DTB, tag="bd", name="bd0", bufs=2)
        # bd0 = sh0W(ws0) * shm0, with the W-shift folded into the read AP
        P.tensor_mul(bd0[:, :, 0:511], ws0[:, :, 1:512], shm0[:, :, 0:511])
        P.tensor_mul(bd0[:, :, 511:512], ws0[:, :, 0:1], shm0[:, :, 511:512])
        V.tensor_max(t_acc[:, :, :], t_acc[:, :, :], bd0[:, :, :])

        # ---- angle 1 (dbta dir6 = H up, blocked dir2 = H down) ----
        isa1 = isa(1)
        msa1 = shp.tile(SHP, DTB, tag="msa", name="msa1", bufs=2)
        V.tensor_mul(msa1[:, :, :], isa1[:, :, :], S0b[:, :, :])
        trail = mp.tile(SHP, DTB, name="trail")
        V.tensor_max(trail[:, :, :], msa0[:, :, :], msa1[:, :, :])
        shm1 = hshift("shm1", msa1, up=True)  # dbta1
        shw1 = hshift("shw1", ws0, up=False)  # blocked1
        bd1 = shp.tile(SHP, DTB, tag="bd", name="bd1", bufs=2)
        P.tensor_mul(bd1[:, :, :], shw1[:, :, :], shm1[:, :, :])
        V.tensor_max(t_acc[:, :, :], t_acc[:, :, :], bd1[:, :, :])
        reset1 = shp.tile(SHP, DTB, tag="rst", name="reset1", bufs=2)
        V.tensor_mul(reset1[:, :, :], shm0[:, :, :], shm1[:, :, :])
        nR1 = shp.tile(SHP, DTB, tag="rst", name="nR1", bufs=2)
        V.tensor_single_scalar(nR1[:, :, :], reset1[:, :, :], 0.0, OP.is_equal)
        S2 = shp.tile(SHP, DTB, tag="Scur", name="S2", bufs=2)
        V.tensor_mul(S2[:, :, :], nR1[:, :, :], S0b[:, :, :])
        dbs = mp.tile(SHP, DTB, name="dbs")
        V.tensor_max(dbs[:, :, :], shm0[:, :, :], shm1[:, :, :])

        # ---- angle 2 (dbta dir0 = W+, blocked dir4 = W-) ----
        isa2 = isa(2)
        msa2 = shp.tile(SHP, DTB, tag="msa", name="msa2", bufs=2)
        V.tensor_mul(msa2[:, :, :], isa2[:, :, :], S2[:, :, :])
        V.tensor_max(trail[:, :, :], trail[:, :, :], msa2[:, :, :])
        shm2 = wshift("shm2", msa2, plus=True)
        ws2 = shp.tile(SHP, DTB, tag="ws", name="ws2", bufs=1)
        V.tensor_add(ws2[:, :, :], Wlb[:, :, :], S2[:, :, :])
        bd2 = shp.tile(SHP, DTB, tag="bd", name="bd2", bufs=2)
        # bd2 = sh4W(ws2) * shm2, W-shift folded into the read AP
        P.tensor_mul(bd2[:, :, 1:512], ws2[:, :, 0:511], shm2[:, :, 1:512])
        P.tensor_mul(bd2[:, :, 0:1], ws2[:, :, 511:512], shm2[:, :, 0:1])
        V.tensor_max(t_acc[:, :, :], t_acc[:, :, :], bd2[:, :, :])
        reset2 = shp.tile(SHP, DTB, tag="rst", name="reset2", bufs=2)
        V.tensor_mul(reset2[:, :, :], dbs[:, :, :], shm2[:, :, :])
        nR2 = shp.tile(SHP, DTB, tag="rst", name="nR2", bufs=2)
        V.tensor_single_scalar(nR2[:, :, :], reset2[:, :, :], 0.0, OP.is_equal)
        S3 = shp.tile(SHP, DTB, tag="Scur", name="S3", bufs=2)
        V.tensor_mul(S3[:, :, :], nR2[:, :, :], S2[:, :, :])
        V.tensor_max(dbs[:, :, :], dbs[:, :, :], shm2[:, :, :])
        dm2 = shp.tile(SHP, DTB, tag="dm", name="dm2", bufs=1)
        V.tensor_scalar_mul(dm2[:, :, :], shm2[:, :, :], 2.0)
        dirc = mp.tile(SHP, DTB, name="dirc")
        V.tensor_max(dirc[:, :, :], dm2[:, :, :], shm1[:, :, :])

        # ---- angle 3 (dbta dir2 = H down, blocked dir6 = H up) ----
        isa3 = isa(3)
        msa3 = shp.tile(SHP, DTB, tag="msa", name="msa3", bufs=2)
        V.tensor_mul(msa3[:, :, :], isa3[:, :, :], S3[:, :, :])
        V.tensor_max(trail[:, :, :], trail[:, :, :], msa3[:, :, :])
        shm3 = hshift("shm3", msa3, up=False)
        ws3 = shp.tile(SHP, DTB, tag="ws", name="ws3", bufs=1)
        V.tensor_add(ws3[:, :, :], Wlb[:, :, :], S3[:, :, :])
        shw3 = hshift("shw3", ws3, up=True)
        bd3 = shp.tile(SHP, DTB, tag="bd", name="bd3", bufs=2)
        P.tensor_mul(bd3[:, :, :], shw3[:, :, :], shm3[:, :, :])
        V.tensor_max(t_acc[:, :, :], t_acc[:, :, :], bd3[:, :, :])
        V.tensor_max(dbs[:, :, :], dbs[:, :, :], shm3[:, :, :])
        dm3 = shp.tile(SHP, DTB, tag="dm", name="dm3", bufs=1)
        V.tensor_scalar_mul(dm3[:, :, :], shm3[:, :, :], 3.0)
        V.tensor_max(dirc[:, :, :], dm3[:, :, :], dirc[:, :, :])

        # ---- finals ----
        TE = shp.tile(SHP, DTB, tag="ptmp", name="TE", bufs=5)
        V.tensor_mul(TE[:, :, :], trail[:, :, :], epos[:, :, :])
        tnE = shp.tile(SHP, DTB, tag="ptmp", name="tnE", bufs=5)
        V.tensor_sub(tnE[:, :, :], trail[:, :, :], TE[:, :, :])
        dbsW = shp.tile(SHP, DTB, tag="ptmp", name="dbsW", bufs=5)
        V.tensor_mul(dbsW[:, :, :], notWl[:, :, :], dbs[:, :, :])
        out_S = mp.tile(SHP, DTB, name="out_S")
        V.tensor_max(out_S[:, :, :], dbsW[:, :, :], tnE[:, :, :])
        out_Sf = shp.tile(SHP, DTF, tag="f32s", name="out_Sf", bufs=3)
        A.copy(out_Sf[:, :, :], out_S[:, :, :])
        SY.dma_start(out=rp(out[3]), in_=out_Sf[:, :, :])
        SW = mp.tile(SHP, DTB, name="SW")  # final wall|snake
        V.tensor_add(SW[:, :, :], out_S[:, :, :], Wlb[:, :, :])

        u = shp.tile(SHP, DTB, tag="ptmp", name="u", bufs=5)
        V.tensor_mul(u[:, :, :], lt05b[:, :, :], TE[:, :, :])
        wE = shp.tile(SHP, DTB, tag="ptmp", name="wE", bufs=5)
        V.tensor_add(wE[:, :, :], u[:, :, :], E0c[:, :, :])
        ndbs = shp.tile(SHP, DTB, tag="ptmp", name="ndbs", bufs=5)
        V.tensor_single_scalar(ndbs[:, :, :], dbs[:, :, :], 0.0, OP.is_equal)
        out_E = shp.tile(SHP, DTB, tag="ptmp", name="out_E", bufs=5)
        V.tensor_mul(out_E[:, :, :], ndbs[:, :, :], wE[:, :, :])
        out_Ef = shp.tile(SHP, DTF, tag="f32s", name="out_Ef", bufs=3)
        A.copy(out_Ef[:, :, :], out_E[:, :, :])
        SY.dma_start(out=rp(out[0]), in_=out_Ef[:, :, :])
        oa1 = shp.tile(SHP, DTB, tag="ptmp", name="oa1", bufs=5)
        V.tensor_sub(oa1[:, :, :], notWl[:, :, :], out_E[:, :, :])
        out_A = shp.tile(SHP, DTB, tag="ptmp", name="out_A", bufs=5)
        V.tensor_sub(out_A[:, :, :], oa1[:, :, :], out_S[:, :, :])
        out_Af = shp.tile(SHP, DTF, tag="f32s", name="out_Af", bufs=3)
        A.copy(out_Af[:, :, :], out_A[:, :, :])
        SY.dma_start(out=rp(out[2]), in_=out_Af[:, :, :])

        # turned = (dirc + 1 - 2*(Re<0.5)) mod 4, branch-free (no HW mod op):
        # x5 in {-1..4}; +4 where x5<0; -4 where >=4.
        x5a = shp.tile(SHP, DTB, tag="ptmp", name="x5a", bufs=5)
        V.tensor_scalar_add(x5a[:, :, :], dirc[:, :, :], 1.0)
        x5 = shp.tile(SHP, DTB, tag="ptmp", name="x5", bufs=5)
        V.tensor_add(x5[:, :, :], x5a[:, :, :], q2[:, :, :])
        c1 = shp.tile(SHP, DTB, tag="ptmp", name="c1", bufs=5)
        V.tensor_scalar(c1[:, :, :], x5[:, :, :], 0.0, 4.0, OP.is_lt, OP.mult)
        y4 = shp.tile(SHP, DTB, tag="ptmp", name="y4", bufs=5)
        V.tensor_add(y4[:, :, :], x5[:, :, :], c1[:, :, :])
        c2 = shp.tile(SHP, DTB, tag="ptmp", name="c2", bufs=5)
        V.tensor_scalar(c2[:, :, :], y4[:, :, :], 4.0, -4.0, OP.is_ge, OP.mult)
        turned = mp.tile(SHP, DTB, name="turned")
        V.tensor_add(turned[:, :, :], y4[:, :, :], c2[:, :, :])

        def mk(k):
            d = shp.tile(SHP, DTB, tag="mk", name=f"m{k}", bufs=1)
            V.tensor_single_scalar(d[:, :, :], turned[:, :, :], float(k), OP.is_equal)
            return d

        sh0SW = wshift("sh0SW", SW, plus=True)
        m0 = mk(0)
        acc = mp.tile(SHP, DTB, name="acc")
        V.tensor_mul(acc[:, :, :], m0[:, :, :], sh0SW[:, :, :])
        SW2 = hshift("SW2", SW, up=False)
        m1 = mk(1)
        tk1 = shp.tile(SHP, DTB, tag="tk", name="tk1", bufs=1)
        V.tensor_mul(tk1[:, :, :], m1[:, :, :], SW2[:, :, :])
        V.tensor_add(acc[:, :, :], acc[:, :, :], tk1[:, :, :])
        sh4SW = wshift("sh4SW", SW, plus=False)
        m2 = mk(2)
        tk2 = shp.tile(SHP, DTB, tag="tk", name="tk2", bufs=1)
        V.tensor_mul(tk2[:, :, :], m2[:, :, :], sh4SW[:, :, :])
        V.tensor_add(acc[:, :, :], acc[:, :, :], tk2[:, :, :])
        SW6 = hshift("SW6", SW, up=True)
        m3 = mk(3)
        tk3 = shp.tile(SHP, DTB, tag="tk", name="tk3", bufs=1)
        V.tensor_mul(tk3[:, :, :], m3[:, :, :], SW6[:, :, :])
        V.tensor_add(acc[:, :, :], acc[:, :, :], tk3[:, :, :])

        # does_turn &= no snake/wall in turned direction (uint8 CP mask)
        nacc = shp.tile(SHP, DTB, tag="ptmp", name="nacc", bufs=5)
        V.tensor_single_scalar(nacc[:, :, :], acc[:, :, :], 0.0, OP.is_equal)
        tU = shp.tile(SHP, DTU8, tag="u8", name="tU", bufs=3)
        V.tensor_mul(tU[:, :, :], nacc[:, :, :], t_acc[:, :, :])
        # dir_came = where(does_turn, turned, dir_came)
        V.copy_predicated(dirc[:, :, :], tU[:, :, :], turned[:, :, :])

        out_Su = shp.tile(SHP, DTU8, tag="u8", name="out_Su", bufs=3)
        A.copy(out_Su[:, :, :], out_S[:, :, :])
        nS0 = shp.tile(SHP, DTB, tag="ptmp", name="nS0", bufs=5)
        V.tensor_single_scalar(nS0[:, :, :], S0b[:, :, :], 0.0, OP.is_equal)
        nb = shp.tile(SHP, DTU8, tag="u8", name="nb", bufs=3)
        V.tensor_mul(nb[:, :, :], nS0[:, :, :], out_S[:, :, :])
        nottrail = shp.tile(SHP, DTB, tag="ptmp", name="nottrail", bufs=5)
        V.tensor_single_scalar(nottrail[:, :, :], trail[:, :, :], 0.0, OP.is_equal)
        out_D = shp.tile(SHP, DTB, tag="ptmp", name="out_D", bufs=5)
        V.tensor_mul(out_D[:, :, :], nottrail[:, :, :], D0b[:, :, :])
        V.copy_predicated(out_D[:, :, :], out_Su[:, :, :], D0b[:, :, :])
        V.copy_predicated(out_D[:, :, :], nb[:, :, :], dirc[:, :, :])
        out_Df = shp.tile(SHP, DTF, tag="f32s", name="out_Df", bufs=3)
        A.copy(out_Df[:, :, :], out_D[:, :, :])
        SY.dma_start(out=rp(out[6]), in_=out_Df[:, :, :])

        out_Enf = shp.tile(SHP, DTF, tag="f32s", name="out_Enf", bufs=3)
        V.tensor_mul(out_Enf[:, :, :], nottrail[:, :, :], En0[:, :, :])
        V.copy_predicated(out_Enf[:, :, :], out_Su[:, :, :], em[:, :, :])
        SY.dma_start(out=rp(out[7]), in_=out_Enf[:, :, :])

        # zero channels + wall passthrough last (lowest DMA priority)
        zero = mp.tile(SHP, DTF, name="zero")
        P.memset(zero[:, :, :], 0.0)
        for c in (4, 5, 8, 9):
            SY.dma_start(out=rp(out[c]), in_=zero[:, :, :])
        SY.dma_start(out=rp(out[1]), in_=Wl_f[:, :, :])


_CACHED = None


def build_program():
    global _CACHED
    if _CACHED is not None:
        return _CACHED
    nc = bacc.Bacc("TRN2", target_bir_lowering=False, debug=False, num_devices=NCORES)
    world_t = nc.dram_tensor("world", [C, H, W], DTF, kind="ExternalInput").ap()
    rm_t = nc.dram_tensor("rm", [H, W], DTF, kind="ExternalInput").ap()
    re_t = nc.dram_tensor("re", [H, W], DTF, kind="ExternalInput").ap()
    shmat_t = nc.dram_tensor("shmat", [128, 512], DTB, kind="ExternalInput").ap()
    out_t = nc.dram_tensor("out", [C, H, W], DTF, kind="ExternalOutput").ap()
    with tile.TileContext(nc) as tc:
        snake_body(
            tc,
            {"out": out_t},
            {"world": world_t, "rm": rm_t, "re": re_t, "shmat": shmat_t},
        )
    nc.compile()
    _CACHED = nc
    return nc


def kernel(**inputs) -> np.ndarray:
    world = np.ascontiguousarray(np.asarray(inputs["world"], dtype=np.float32))
    rmov = np.ascontiguousarray(np.asarray(inputs["rand_movement"], dtype=np.float32))
    rele = np.ascontiguousarray(np.asarray(inputs["rand_element"], dtype=np.float32))
    shmat = make_shmat()

    nc = build_program()
    in_maps = [
        {
            "world": world[b],
            "rm": rmov[b, 0],
            "re": rele[b, 0],
            "shmat": shmat,
        }
        for b in range(B)
    ]
    res = bass_utils.run_bass_kernel_spmd(nc, in_maps, core_ids=list(range(NCORES)))
    return np.stack([res.results[b]["out"] for b in range(B)], axis=0)

